# revision 26
# baseline (speedup 1.0000x reference)
"""Trainium2 Bass kernel for nn_DiscreteExactLoss (joint-entropy loss).

Reference computation:
    soft_assign[b, r, :] = [1 - a[b,r], a[b,r]]          (K=2, R=10)
    joint_p[b, s]  = prod_r soft_assign[b, r, s_r]       (s in [0, 1024))
    p_a            = mean_b joint_p                       [1024]
    out            = sum_s p_a * log2(p_a)               (scalar, ~-10)

Device algorithm (per core, data-parallel over B across 8 cores):
    Accumulate MULTILINEAR MOMENTS m_T = sum_b prod_{r in T} a[b, r] for
    all 1024 subsets T via a 5+5 variable split: per sample, two 32-entry
    subset-product tables (A half = vars 0-4, C half = vars 5-9) built by
    doubling, and cross moments = sum_b MA[b] (x) MC[b] on the
    TensorEngine with contraction over samples.

    Samples are organized as 128 chunks of 128 (partition = position
    within chunk). Four chunks are packed per matmul: lhsT/rhs are
    [128, 128] (cols = 32 table entries x 4 chunks) so there are only 32
    LDWEIGHTS+MATMUL pairs, weights are contiguous (FWL-eligible), and
    N=128 per MM. The [128,128] PSUM accumulator holds a 4x4 grid of
    32x32 blocks of which only the 4 diagonal ones (cg==cg') matter; the
    host extracts and sums them.

    Pipeline over 6 tiles of [12,28,32,32,16,8] chunks: DMA (HWDGE) ->
    cast + transpose to chunk-innermost bf16 (tile 0 on DVE so it isn't
    gated on the ACT activation-table load; tiles 1+ on the Scalar
    engine, overlapped) -> DVE doubling build (2x-mode tensor_tensor,
    (h g)-merged 3-free-dim APs) -> PE matmuls. The shrinking tail
    tiles (16, 8 chunks) keep the final DVE->matmul dependency short.
    Measured structure: ~6.6us fixed framework preamble, first-DMA
    completion semaphore at ~8.9us (HBM receipt latency), DVE saturated
    to ~16.9us, last matmul ~17.6us, then a fixed ~4.5us tail (PSUM
    copy + output DMA + receipt + framework semaphore-sweep).

    Host side: sum the 8 per-core partials, apply the tiny Mobius
    transform (moments -> probabilities, 10 butterfly stages over a
    1024-vector), then p*log2(p) reduction (~30k flops, negligible).
"""

import math
import sys

import numpy as np

if "/opt/trn_rl_repo" not in sys.path:
    sys.path.insert(0, "/opt/trn_rl_repo")

B_FULL = 131072
R_FULL = 10
N_CORES = 8
B_LOC = B_FULL // N_CORES  # 16384
P = 128                    # SBUF partitions; samples per chunk
C = B_LOC // P             # 128 chunks per core

TILE_CHUNKS = [12, 28, 32, 32, 20, 4]   # chunks per DVE pipeline tile
N_WARM = 0                           # PE warm-up matmuls (cadence is
                                     # LDWEIGHTS-bound, so warm-up is moot)

_NC_CACHE = {}


def _build_module():
    if "nc" in _NC_CACHE:
        return _NC_CACHE["nc"]

    from concourse import bacc, bass, mybir, tile

    f32 = mybir.dt.float32
    bf16 = mybir.dt.bfloat16

    nc = bacc.Bacc("TRN2", target_bir_lowering=False, debug=False)

    act = nc.dram_tensor("act", [B_LOC, R_FULL], f32, kind="ExternalInput")
    msum = nc.dram_tensor("msum", [P, P], f32, kind="ExternalOutput")

    # dram view [p, c, r]: sample b = p*C + c
    act_pcr = act.ap().rearrange("(p c) r -> p c r", p=P)

    n_groups_total = sum(TILE_CHUNKS) // 4

    with tile.TileContext(nc) as tc:
        with (
            tc.tile_pool(name="a0", bufs=5) as a0_pool,
            tc.tile_pool(name="avd", bufs=2) as avd_pool,
            tc.tile_pool(name="tabd", bufs=2) as tabd_pool,
            tc.tile_pool(name="warm", bufs=1) as warm_pool,
            tc.tile_pool(name="outp", bufs=1) as out_pool,
            tc.tile_pool(name="psum", bufs=1, space=bass.MemorySpace.PSUM) as psum_pool,
            tc.tile_pool(name="psumw", bufs=1, space=bass.MemorySpace.PSUM) as psumw_pool,
        ):
            psum_acc = psum_pool.tile([P, P], f32)

            # ---- optional PE warm-up matmuls (HAM clock-gate). Measured
            # moot here: MM issue cadence is LDWEIGHTS-bound (~107 ns,
            # NX path at fixed 1.2 GHz), not PE-clock-bound.
            if N_WARM:
                warm_sb = warm_pool.tile([P, P], bf16)
                warm_ps = psumw_pool.tile([P, P], f32)
                nc.vector.memset(warm_sb[:, :], 0.0)
                for _ in range(N_WARM):
                    nc.tensor.matmul(
                        warm_ps[:, :], warm_sb[:, :], warm_sb[:, :],
                        start=True, stop=True,
                    )

            gg = 0
            c_base = 0
            for t, tc_chunks in enumerate(TILE_CHUNKS):
                ng = tc_chunks // 4

                # ---- load raw fp32 activity for this tile's chunks ----
                a0 = a0_pool.tile([P, tc_chunks, R_FULL], f32, tag=f"a0{t}")
                nc.sync.dma_start(
                    out=a0[:, :, :],
                    in_=act_pcr[:, c_base:c_base + tc_chunks, :],
                )

                # ---- cast f32->bf16 + transpose to chunk-innermost ----
                # avar[p, l, h, c]: var r = h*5 + l of chunk c. Tile 0 on
                # DVE (not gated on the ACT table load; also keeps DVE
                # busy until tile 1's transpose lands); tiles 1+ on the
                # Scalar engine.
                a0_lhc = a0.rearrange("p c (h l) -> p l h c", h=2)
                avar = avd_pool.tile([P, 5, 2, tc_chunks], bf16, tag=f"av{t}")
                if t == 0:
                    nc.vector.tensor_copy(avar[:, :, :, :], a0_lhc)
                else:
                    nc.scalar.copy(avar[:, :, :, :], a0_lhc)

                # ---- DVE: subset-product tables by doubling ----
                # tab[p, h, g, i, cg]; ops use (h g)-merged 3-free-dim APs.
                tab = tabd_pool.tile([P, 2, ng, 32, 4], bf16, tag=f"td{t}")

                def tabv(lo, hi, tab=tab, ng=ng):
                    return tab[:, :, :, lo:hi, :].rearrange(
                        "p h g i c -> p (h g) i c"
                    )

                def avarv(lvl, avar=avar, ng=ng):
                    return avar[:, lvl, :, :].rearrange(
                        "p h (g c) -> p (h g) c", g=ng
                    ).unsqueeze(2)

                nc.vector.memset(tabv(0, 1), 1.0)
                nc.vector.tensor_copy(tabv(1, 2), avarv(0))
                for lvl in range(1, 5):
                    j = 1 << lvl
                    nc.vector.tensor_tensor(
                        tabv(j, 2 * j),
                        tabv(0, j),
                        avarv(lvl).broadcast_to([P, 2 * ng, j, 4]),
                        mybir.AluOpType.mult,
                    )

                # ---- PE: accumulate sum_b MA (x) MC, 4 chunks per MM ----
                for g in range(ng):
                    nc.tensor.matmul(
                        psum_acc[:, :],
                        tab[:, 0, g, :, :].rearrange("p i c -> p (i c)"),
                        tab[:, 1, g, :, :].rearrange("p i c -> p (i c)"),
                        start=(gg == 0),
                        stop=(gg == n_groups_total - 1),
                    )
                    gg += 1
                c_base += tc_chunks

            out_sb = out_pool.tile([P, P], f32)
            nc.vector.tensor_copy(out_sb[:, :], psum_acc[:, :])
            nc.sync.dma_start(out=msum[:, :], in_=out_sb[:, :])

    # Bacc modules carry virtual registers until compile() runs; the
    # bass2jax/PJRT path serializes nc as-is, so allocate them now.
    nc.compile()
    _NC_CACHE["nc"] = nc
    return nc


def _ensure_ntff_hook():
    """The agent image's antenv package lacks axon_hooks; synthesize it so
    run_bass_kernel_spmd(trace=True) can find the NTFF profile hook."""
    import types

    try:
        from antenv.axon_hooks import get_axon_ntff_profile_hook  # noqa: F401
        return
    except ImportError:
        pass
    import antenv

    mod = types.ModuleType("antenv.axon_hooks")
    state = {"hook": None}
    mod.set_axon_ntff_profile_hook = lambda h: state.__setitem__("hook", h)
    mod.get_axon_ntff_profile_hook = lambda: state["hook"]
    antenv.axon_hooks = mod
    sys.modules["antenv.axon_hooks"] = mod

    try:
        from trn_agent_boot.trn_boot import _ntff_profile_via_ctypes

        hook = _ntff_profile_via_ctypes("/opt/axon/libaxon_pjrt.so")
        if hook is not None:
            mod.set_axon_ntff_profile_hook(hook)
    except Exception:
        pass


def _run_on_device(activity, trace=False):
    from concourse.bass_utils import run_bass_kernel_spmd

    if trace:
        _ensure_ntff_hook()
    nc = _build_module()
    shards = np.ascontiguousarray(activity.astype(np.float32)).reshape(
        N_CORES, B_LOC, R_FULL
    )
    in_maps = [{"act": np.ascontiguousarray(shards[i])} for i in range(N_CORES)]
    res = run_bass_kernel_spmd(
        nc, in_maps, core_ids=list(range(N_CORES)), trace=trace
    )
    return res


def _finish_on_host(per_core_msums):
    # total moment sums over all B samples; psum is a 4x4 grid of 32x32
    # blocks (m=(i,cg), n=(j,cg')) of which the cg==cg' diagonal holds
    # per-chunk-group moment partials.
    acc = np.zeros((P, P), dtype=np.float64)
    for part in per_core_msums:
        acc += part.astype(np.float64)
    p4 = acc.reshape(32, 4, 32, 4)
    msum = sum(p4[:, k, :, k] for k in range(4))
    m = (msum / B_FULL).reshape(-1)  # [1024] mean moments

    # Mobius transform per bit: p(bit=0) = m(without) - m(with)
    p = m.copy()
    idx = np.arange(1024)
    for bit in range(10):
        step = 1 << bit
        lo = idx[(idx & step) == 0]
        p[lo] = p[lo] - p[lo | step]

    p = p.astype(np.float32)
    p_safe = np.clip(p, 1e-12, None)
    log_k_p = np.log(p_safe) / math.log(2.0)
    joint_h = -np.sum(p * log_k_p)
    return np.array(-joint_h, dtype=np.float32)


def kernel(activity):
    res = _run_on_device(activity, trace=False)
    return _finish_on_host([r["msum"] for r in res.results])


def kernel_profiled(activity):
    """Like kernel() but with NTFF tracing; returns (output, exec_time_ns)."""
    res = _run_on_device(activity, trace=True)
    out = _finish_on_host([r["msum"] for r in res.results])
    return out, res.exec_time_ns


# revision 27
# speedup vs baseline: 1.0452x; 1.0452x over previous
"""Trainium2 Bass kernel for nn_DiscreteExactLoss (joint-entropy loss).

Reference computation:
    soft_assign[b, r, :] = [1 - a[b,r], a[b,r]]          (K=2, R=10)
    joint_p[b, s]  = prod_r soft_assign[b, r, s_r]       (s in [0, 1024))
    p_a            = mean_b joint_p                       [1024]
    out            = sum_s p_a * log2(p_a)               (scalar, ~-10)

Device algorithm (per core, data-parallel over B across 8 cores):
    Accumulate MULTILINEAR MOMENTS m_T = sum_b prod_{r in T} a[b, r] for
    all 1024 subsets T via a 5+5 variable split: per sample, two 32-entry
    subset-product tables (A half = vars 0-4, C half = vars 5-9) built by
    doubling, and cross moments = sum_b MA[b] (x) MC[b] on the
    TensorEngine with contraction over samples.

    Samples are organized as 128 chunks of 128 (partition = position
    within chunk). Four chunks are packed per matmul: lhsT/rhs are
    [128, 128] (cols = 32 table entries x 4 chunks) so there are only 32
    LDWEIGHTS+MATMUL pairs, weights are contiguous (FWL-eligible), and
    N=128 per MM. The [128,128] PSUM accumulator holds a 4x4 grid of
    32x32 blocks of which only the 4 diagonal ones (cg==cg') matter; the
    host extracts and sums them.

    Pipeline over 6 tiles of [12,28,32,32,16,8] chunks: DMA (HWDGE) ->
    cast + transpose to chunk-innermost bf16 (tile 0 on DVE so it isn't
    gated on the ACT activation-table load; tiles 1+ on the Scalar
    engine, overlapped) -> DVE doubling build (2x-mode tensor_tensor,
    (h g)-merged 3-free-dim APs) -> PE matmuls. The shrinking tail
    tiles (16, 8 chunks) keep the final DVE->matmul dependency short.
    Measured structure: ~6.6us fixed framework preamble, first-DMA
    completion semaphore at ~8.9us (HBM receipt latency), DVE saturated
    to ~16.9us, last matmul ~17.6us, then a fixed ~4.5us tail (PSUM
    copy + output DMA + receipt + framework semaphore-sweep).

    Host side: sum the 8 per-core partials, apply the tiny Mobius
    transform (moments -> probabilities, 10 butterfly stages over a
    1024-vector), then p*log2(p) reduction (~30k flops, negligible).
"""

import math
import sys

import numpy as np

if "/opt/trn_rl_repo" not in sys.path:
    sys.path.insert(0, "/opt/trn_rl_repo")

B_FULL = 131072
R_FULL = 10
N_CORES = 8
B_LOC = B_FULL // N_CORES  # 16384
P = 128                    # SBUF partitions; samples per chunk
C = B_LOC // P             # 128 chunks per core

TILE_CHUNKS = [12, 28, 32, 32, 16, 8]   # chunks per DVE pipeline tile
N_WARM = 0                           # PE warm-up matmuls (cadence is
                                     # LDWEIGHTS-bound, so warm-up is moot)

_NC_CACHE = {}


def _build_module():
    if "nc" in _NC_CACHE:
        return _NC_CACHE["nc"]

    from concourse import bacc, bass, mybir, tile

    f32 = mybir.dt.float32
    bf16 = mybir.dt.bfloat16

    nc = bacc.Bacc("TRN2", target_bir_lowering=False, debug=False)

    act = nc.dram_tensor("act", [B_LOC, R_FULL], f32, kind="ExternalInput")
    msum = nc.dram_tensor("msum", [P, P], f32, kind="ExternalOutput")

    # dram view [p, c, r]: sample b = p*C + c
    act_pcr = act.ap().rearrange("(p c) r -> p c r", p=P)

    n_groups_total = sum(TILE_CHUNKS) // 4

    with tile.TileContext(nc) as tc:
        with (
            tc.tile_pool(name="a0", bufs=5) as a0_pool,
            tc.tile_pool(name="avd", bufs=2) as avd_pool,
            tc.tile_pool(name="tabd", bufs=2) as tabd_pool,
            tc.tile_pool(name="warm", bufs=1) as warm_pool,
            tc.tile_pool(name="outp", bufs=1) as out_pool,
            tc.tile_pool(name="psum", bufs=1, space=bass.MemorySpace.PSUM) as psum_pool,
            tc.tile_pool(name="psumw", bufs=1, space=bass.MemorySpace.PSUM) as psumw_pool,
        ):
            psum_acc = psum_pool.tile([P, P], f32)

            # ---- optional PE warm-up matmuls (HAM clock-gate). Measured
            # moot here: MM issue cadence is LDWEIGHTS-bound (~107 ns,
            # NX path at fixed 1.2 GHz), not PE-clock-bound.
            if N_WARM:
                warm_sb = warm_pool.tile([P, P], bf16)
                warm_ps = psumw_pool.tile([P, P], f32)
                nc.vector.memset(warm_sb[:, :], 0.0)
                for _ in range(N_WARM):
                    nc.tensor.matmul(
                        warm_ps[:, :], warm_sb[:, :], warm_sb[:, :],
                        start=True, stop=True,
                    )

            gg = 0
            c_base = 0
            for t, tc_chunks in enumerate(TILE_CHUNKS):
                ng = tc_chunks // 4

                # ---- load raw fp32 activity for this tile's chunks ----
                a0 = a0_pool.tile([P, tc_chunks, R_FULL], f32, tag=f"a0{t}")
                nc.sync.dma_start(
                    out=a0[:, :, :],
                    in_=act_pcr[:, c_base:c_base + tc_chunks, :],
                )

                # ---- cast f32->bf16 + transpose to chunk-innermost ----
                # avar[p, l, h, c]: var r = h*5 + l of chunk c. Tile 0 on
                # DVE (not gated on the ACT table load; also keeps DVE
                # busy until tile 1's transpose lands); tiles 1+ on the
                # Scalar engine.
                a0_lhc = a0.rearrange("p c (h l) -> p l h c", h=2)
                avar = avd_pool.tile([P, 5, 2, tc_chunks], bf16, tag=f"av{t}")
                if t == 0:
                    nc.vector.tensor_copy(avar[:, :, :, :], a0_lhc)
                else:
                    nc.scalar.copy(avar[:, :, :, :], a0_lhc)

                # ---- DVE: subset-product tables by doubling ----
                # tab[p, h, g, i, cg]; ops use (h g)-merged 3-free-dim APs.
                tab = tabd_pool.tile([P, 2, ng, 32, 4], bf16, tag=f"td{t}")

                def tabv(lo, hi, tab=tab, ng=ng):
                    return tab[:, :, :, lo:hi, :].rearrange(
                        "p h g i c -> p (h g) i c"
                    )

                def avarv(lvl, avar=avar, ng=ng):
                    return avar[:, lvl, :, :].rearrange(
                        "p h (g c) -> p (h g) c", g=ng
                    ).unsqueeze(2)

                nc.vector.memset(tabv(0, 1), 1.0)
                nc.vector.tensor_copy(tabv(1, 2), avarv(0))
                for lvl in range(1, 5):
                    j = 1 << lvl
                    nc.vector.tensor_tensor(
                        tabv(j, 2 * j),
                        tabv(0, j),
                        avarv(lvl).broadcast_to([P, 2 * ng, j, 4]),
                        mybir.AluOpType.mult,
                    )

                # ---- PE: accumulate sum_b MA (x) MC, 4 chunks per MM ----
                for g in range(ng):
                    nc.tensor.matmul(
                        psum_acc[:, :],
                        tab[:, 0, g, :, :].rearrange("p i c -> p (i c)"),
                        tab[:, 1, g, :, :].rearrange("p i c -> p (i c)"),
                        start=(gg == 0),
                        stop=(gg == n_groups_total - 1),
                    )
                    gg += 1
                c_base += tc_chunks

            out_sb = out_pool.tile([P, P], f32)
            nc.vector.tensor_copy(out_sb[:, :], psum_acc[:, :])
            nc.sync.dma_start(out=msum[:, :], in_=out_sb[:, :])

    # Bacc modules carry virtual registers until compile() runs; the
    # bass2jax/PJRT path serializes nc as-is, so allocate them now.
    nc.compile()
    _NC_CACHE["nc"] = nc
    return nc


def _ensure_ntff_hook():
    """The agent image's antenv package lacks axon_hooks; synthesize it so
    run_bass_kernel_spmd(trace=True) can find the NTFF profile hook."""
    import types

    try:
        from antenv.axon_hooks import get_axon_ntff_profile_hook  # noqa: F401
        return
    except ImportError:
        pass
    import antenv

    mod = types.ModuleType("antenv.axon_hooks")
    state = {"hook": None}
    mod.set_axon_ntff_profile_hook = lambda h: state.__setitem__("hook", h)
    mod.get_axon_ntff_profile_hook = lambda: state["hook"]
    antenv.axon_hooks = mod
    sys.modules["antenv.axon_hooks"] = mod

    try:
        from trn_agent_boot.trn_boot import _ntff_profile_via_ctypes

        hook = _ntff_profile_via_ctypes("/opt/axon/libaxon_pjrt.so")
        if hook is not None:
            mod.set_axon_ntff_profile_hook(hook)
    except Exception:
        pass


def _run_on_device(activity, trace=False):
    from concourse.bass_utils import run_bass_kernel_spmd

    if trace:
        _ensure_ntff_hook()
    nc = _build_module()
    shards = np.ascontiguousarray(activity.astype(np.float32)).reshape(
        N_CORES, B_LOC, R_FULL
    )
    in_maps = [{"act": np.ascontiguousarray(shards[i])} for i in range(N_CORES)]
    res = run_bass_kernel_spmd(
        nc, in_maps, core_ids=list(range(N_CORES)), trace=trace
    )
    return res


def _finish_on_host(per_core_msums):
    # total moment sums over all B samples; psum is a 4x4 grid of 32x32
    # blocks (m=(i,cg), n=(j,cg')) of which the cg==cg' diagonal holds
    # per-chunk-group moment partials.
    acc = np.zeros((P, P), dtype=np.float64)
    for part in per_core_msums:
        acc += part.astype(np.float64)
    p4 = acc.reshape(32, 4, 32, 4)
    msum = sum(p4[:, k, :, k] for k in range(4))
    m = (msum / B_FULL).reshape(-1)  # [1024] mean moments

    # Mobius transform per bit: p(bit=0) = m(without) - m(with)
    p = m.copy()
    idx = np.arange(1024)
    for bit in range(10):
        step = 1 << bit
        lo = idx[(idx & step) == 0]
        p[lo] = p[lo] - p[lo | step]

    p = p.astype(np.float32)
    p_safe = np.clip(p, 1e-12, None)
    log_k_p = np.log(p_safe) / math.log(2.0)
    joint_h = -np.sum(p * log_k_p)
    return np.array(-joint_h, dtype=np.float32)


def kernel(activity):
    res = _run_on_device(activity, trace=False)
    return _finish_on_host([r["msum"] for r in res.results])


def kernel_profiled(activity):
    """Like kernel() but with NTFF tracing; returns (output, exec_time_ns)."""
    res = _run_on_device(activity, trace=True)
    out = _finish_on_host([r["msum"] for r in res.results])
    return out, res.exec_time_ns
